# revision 14
# baseline (speedup 1.0000x reference)
"""Trainium2 Bass kernel for an LSTM (B=256, T=512, F=128, H=512, OUT=128).

Strategy (data-parallel over 8 cores, B_local=32 per core):
  - Recurrent gates computed in a "folded" PSUM layout [128, 512]:
    partition = 32*quadrant + batch, quadrants = (i, f, g, o), free = hidden
    unit. Each gate quadrant is produced by a column-tiled matmul
    (tile_position=(0, 32*q)) so the 4 quadrants stream concurrently
    through the PE array. Operands are bitcast to float32r (full PE rate
    at N=512 with near-fp32 precision; PSUM accumulation stays fp32).
  - Per step, one PSUM accumulation per quadrant: bias (K=1 ones-row
    matmul) + W_i @ x_t (K=128) + 4x W_h chunks (K=128 each).
  - One sigmoid ACT pass over all 128 partitions with a per-partition
    scale vector (2.0 on the g quadrant): sigmoid(2a) = (tanh(a)+1)/2, so
    g = 2*sig - 1 via a single DVE tensor_scalar.
  - c update via a fused 64-channel DVE product ([i;f] * [g;c]) + add.
  - h built in a "folded" [128, 128] layout (partition = 32*chunk + batch)
    so the 4 PE transposes target distinct row groups and run
    concurrently; transposed h feeds next step's stationary operands.
  - Weights stay resident in SBUF; x is pre-transposed on the host to
    [F, T, B_local] and streamed in 16-step chunks.
"""

import numpy as np

N_FEATURES = 128
N_HIDDEN = 512
N_OUTPUT = 128
BATCH = 256
TIME = 512
N_CORES = 8
B_LOC = BATCH // N_CORES  # 32
XCHUNK = 16  # timesteps per x DMA chunk

_CACHE = {}


def _build(T, T_data=None):
    import concourse.bass as bass
    import concourse.tile as tile
    from concourse import bacc, mybir

    f32 = mybir.dt.float32
    f16 = mybir.dt.float16
    AF = mybir.ActivationFunctionType
    OP = mybir.AluOpType

    nc = bacc.Bacc(
        "TRN2",
        target_bir_lowering=False,
        debug=False,
        enable_asserts=False,
        num_devices=N_CORES,
    )

    xT_d = nc.dram_tensor("xT", [128, T_data or T, B_LOC], f16, kind="ExternalInput").ap()
    wh_d = nc.dram_tensor("whT", [4, 128, 4, 512], f16, kind="ExternalInput").ap()
    wi_d = nc.dram_tensor("wiT", [128, 4, 512], f16, kind="ExternalInput").ap()
    bias_d = nc.dram_tensor("biasf", [128, 4, 512], f16, kind="ExternalInput").ap()
    wlin_d = nc.dram_tensor("wlinT", [4, 128, 128], f16, kind="ExternalInput").ap()
    blin_d = nc.dram_tensor("blin", [128, 128], f16, kind="ExternalInput").ap()
    ones_d = nc.dram_tensor("ones", [128, B_LOC], f16, kind="ExternalInput").ap()
    id_d = nc.dram_tensor("ident", [128, 32], f16, kind="ExternalInput").ap()
    sc_d = nc.dram_tensor("sigscale", [128, 1], f32, kind="ExternalInput").ap()
    y_d = nc.dram_tensor("y", [B_LOC, 128], f32, kind="ExternalOutput").ap()
    import os
    dbg = os.environ.get("KDBG") == "1"
    if dbg:
        dsig_d = nc.dram_tensor("dsig", [128, 512], f32, kind="ExternalOutput").ap()
        dst_d = nc.dram_tensor("dst", [128, 1536], f32, kind="ExternalOutput").ap()
        dht_d = nc.dram_tensor("dht", [128, 128], f32, kind="ExternalOutput").ap()
        dg_d = nc.dram_tensor("dg", [128, 512], f32, kind="ExternalOutput").ap()

    XC = min(XCHUNK, T)
    n_chunks = (T + XC - 1) // XC

    with tile.TileContext(nc) as tc:
        from contextlib import ExitStack

        with ExitStack() as ctx:
            const = ctx.enter_context(tc.tile_pool(name="const", bufs=1))
            xpool = ctx.enter_context(tc.tile_pool(name="xin", bufs=3))
            sigp = ctx.enter_context(tc.tile_pool(name="sig", bufs=2))
            hfp = ctx.enter_context(tc.tile_pool(name="hfold", bufs=2))
            hTp = ctx.enter_context(tc.tile_pool(name="hT", bufs=2))
            gpsum = ctx.enter_context(tc.tile_pool(name="gates", bufs=3, space="PSUM"))
            tpsum = ctx.enter_context(tc.tile_pool(name="tp", bufs=2, space="PSUM"))
            ypsum = ctx.enter_context(tc.tile_pool(name="yps", bufs=1, space="PSUM"))
            ysb = ctx.enter_context(tc.tile_pool(name="ysb", bufs=1))

            # --- resident constants/weights ---
            wh = const.tile([128, 4, 4, 512], f16)  # [p, kc, jq, u]
            for kc in range(4):
                nc.sync.dma_start(out=wh[:, kc, :, :], in_=wh_d[kc])
            wi = const.tile([128, 4, 512], f16)
            nc.sync.dma_start(out=wi[:], in_=wi_d)
            biasf = const.tile([128, 4, 512], f16)
            nc.sync.dma_start(out=biasf[:], in_=bias_d)
            wlin = const.tile([128, 4, 128], f16)
            for kc in range(4):
                nc.sync.dma_start(out=wlin[:, kc, :], in_=wlin_d[kc])
            blin = const.tile([128, 128], f16)
            nc.sync.dma_start(out=blin[:], in_=blin_d)
            ones = const.tile([128, B_LOC], f16)
            nc.sync.dma_start(out=ones[:], in_=ones_d)
            ident = const.tile([128, 32], f16)
            nc.sync.dma_start(out=ident[:], in_=id_d)
            sscale = const.tile([128, 1], f32)
            nc.sync.dma_start(out=sscale[:], in_=sc_d)

            # persistent state tile. Layout (verifier requires equal base
            # partitions for the two tensor_tensor inputs, so pairs are
            # co-located by partition base and separated in the free dim):
            #   ST[0:32,   0:512]    g' = 2*sig(2a_g)-1   (pairs with i @0)
            #   ST[32:64,  0:512]    c  (persistent)      (pairs with f @32)
            #   ST[32:64,  512:1024] p1 = i*g'
            #   ST[32:64, 1024:1536] p2 = f*c
            #   ST[96:128, 0:512]    tanh(c)              (pairs with o @96)
            ST = const.tile([128, 1536], f32)
            nc.vector.memset(ST[:], 0.0)

            # x chunks
            xc = {}

            def load_chunk(c):
                if c < n_chunks and c not in xc:
                    xc[c] = xpool.tile([128, XC, B_LOC], f16, name="xc", tag="xc")
                    t0 = c * XC
                    nc.sync.dma_start(out=xc[c][:], in_=xT_d[:, t0 : t0 + XC, :])

            load_chunk(0)
            load_chunk(1)

            def emit_bias_x(g_ps, t, is_last_of_group):
                # bias row (K=1) : only the FIRST matmul of the tile's
                # accumulation group carries start=True (it clears the whole
                # bank's has_written bits).
                for jq in range(4):
                    nc.tensor.matmul(
                        g_ps[32 * jq : 32 * jq + 32, :],
                        ones[:],
                        biasf[:, jq, :],
                        start=True,
                        stop=False,
                        tile_position=(0, 32 * jq),
                        skip_group_check=True,
                    )
                xt = xc[t // XC][:, t % XC, :]
                for jq in range(4):
                    nc.tensor.matmul(
                        g_ps[32 * jq : 32 * jq + 32, :],
                        xt,
                        wi[:, jq, :],
                        start=False,
                        stop=is_last_of_group and (jq == 3),
                        tile_position=(0, 32 * jq),
                        skip_group_check=True,
                    )

            # t=0 has no h matmuls (h_0 = 0), so its group ends at the x MMs
            g_ps = gpsum.tile([128, 512], f32, name="g", tag="g")
            emit_bias_x(g_ps, 0, True)
            hT_prev = None

            for t in range(T):
                if t > 0:
                    for kc in range(4):
                        for jq in range(4):
                            nc.tensor.matmul(
                                g_ps[32 * jq : 32 * jq + 32, :],
                                hT_prev[:, 32 * kc : 32 * kc + 32],
                                wh[:, kc, jq, :],
                                start=False,
                                stop=(kc == 3 and jq == 3),
                                tile_position=(0, 32 * jq),
                                skip_group_check=True,
                            )

                sig = sigp.tile([128, 512], f32)
                if dbg and t == 0:
                    dgt = sigp.tile([128, 512], f32, name="dgt")
                    nc.vector.tensor_copy(dgt[:], g_ps[:])
                    nc.sync.dma_start(out=dg_d, in_=dgt[:])
                nc.scalar.activation(sig[:], g_ps[:], AF.Sigmoid, scale=sscale[:])
                if dbg and t == 0:
                    nc.sync.dma_start(out=dsig_d, in_=sig[:])

                # g = 2*sig(2a_g) - 1  (tanh via sigmoid identity)
                nc.vector.tensor_scalar(
                    ST[0:32, 0:512], sig[64:96, :], 2.0, -1.0, OP.mult, OP.add
                )
                # p1 = i * g'
                nc.vector.tensor_tensor(
                    ST[32:64, 512:1024], sig[0:32, :], ST[0:32, 0:512], OP.mult
                )
                # p2 = f * c
                nc.vector.tensor_tensor(
                    ST[32:64, 1024:1536], sig[32:64, :], ST[32:64, 0:512], OP.mult
                )
                # c = p1 + p2  (in place)
                nc.vector.tensor_tensor(
                    ST[32:64, 0:512], ST[32:64, 512:1024], ST[32:64, 1024:1536], OP.add
                )
                # tanh(c) -> partitions 96:128 to pair with o
                nc.scalar.activation(ST[96:128, 0:512], ST[32:64, 0:512], AF.Tanh)

                # h folded: partition 32*jc + b, free = unit within chunk jc
                hf = hfp.tile([128, 128], f16)
                for jc in range(4):
                    nc.vector.tensor_tensor(
                        hf[32 * jc : 32 * jc + 32, :],
                        sig[96:128, 128 * jc : 128 * jc + 128],
                        ST[96:128, 128 * jc : 128 * jc + 128],
                        OP.mult,
                    )

                # prefetch x + emit next step's bias/x matmuls before the
                # transposes so the PE can fill the wait on hf
                if t + 1 < T:
                    if (t + 1) % XC == 0:
                        load_chunk((t + 1) // XC + 1)
                    g_next = gpsum.tile([128, 512], f32, name="g", tag="g")
                    emit_bias_x(g_next, t + 1, False)
                else:
                    g_next = None

                # 4 concurrent transposes (distinct row groups)
                tp = tpsum.tile([128, 128], f16)
                for jc in range(4):
                    nc.tensor.transpose(
                        tp[:, 32 * jc : 32 * jc + 32],
                        hf[32 * jc : 32 * jc + 32, :],
                        ident[32 * jc : 32 * jc + 32, :],
                        tile_position=(32 * jc, 0),
                    )
                hT = hTp.tile([128, 128], f16)
                nc.vector.tensor_copy(hT[:], tp[:])

                if dbg and t == 0:
                    nc.sync.dma_start(out=dst_d, in_=ST[:])
                    dhtc = hTp.tile([128, 128], f32, name="dhtc")
                    nc.vector.tensor_copy(dhtc[:], hT[:])
                    nc.sync.dma_start(out=dht_d, in_=dhtc[:])
                hT_prev = hT
                g_ps = g_next

            # final linear: y = h_T @ W_lin.T + b_lin
            y_ps = ypsum.tile([B_LOC, 128], f32)
            nc.tensor.matmul(
                y_ps[:],
                ones[:],
                blin[:],
                start=True,
                stop=False,
                skip_group_check=True,
            )
            for kc in range(4):
                nc.tensor.matmul(
                    y_ps[:],
                    hT_prev[:, 32 * kc : 32 * kc + 32],
                    wlin[:, kc, :],
                    start=False,
                    stop=(kc == 3),
                    skip_group_check=True,
                )
            y_s = ysb.tile([B_LOC, 128], f32)
            nc.vector.tensor_copy(y_s[:], y_ps[:])
            nc.sync.dma_start(out=y_d, in_=y_s[:])

    nc.compile()
    return nc


def _prep_shared(W_i, b_i, W_h, b_h, W_lin, b_lin):
    H = N_HIDDEN
    wh_f = np.empty((4, 128, 4, 512), np.float32)
    for kc in range(4):
        for jq in range(4):
            wh_f[kc, :, jq, :] = W_h[512 * jq : 512 * (jq + 1), 128 * kc : 128 * (kc + 1)].T
    wi_f = np.empty((128, 4, 512), np.float32)
    for jq in range(4):
        wi_f[:, jq, :] = W_i[512 * jq : 512 * (jq + 1), :].T
    bias_f = np.broadcast_to((b_i + b_h).astype(np.float32).reshape(1, 4, 512), (128, 4, 512)).copy()
    wlin_f = np.empty((4, 128, 128), np.float32)
    for kc in range(4):
        wlin_f[kc] = W_lin[:, 128 * kc : 128 * (kc + 1)].T
    blin_f = np.broadcast_to(b_lin.astype(np.float32).reshape(1, 128), (128, 128)).copy()
    ones = np.zeros((128, B_LOC), np.float32)
    ones[0, :] = 1.0  # row-selector: ones.T @ rhs == rhs[0, :] broadcast
    ident = np.tile(np.eye(32, dtype=np.float32), (4, 1))
    sscale = np.ones((128, 1), np.float32)
    sscale[64:96] = 2.0
    h16 = np.float16
    return dict(
        whT=wh_f.astype(h16), wiT=wi_f.astype(h16), biasf=bias_f.astype(h16),
        wlinT=wlin_f.astype(h16), blin=blin_f.astype(h16), ones=ones.astype(h16),
        ident=ident.astype(h16), sigscale=sscale,
    )


def _run(nc, in_maps, want_trace=False):
    from concourse.bass_utils import run_bass_kernel_spmd

    return run_bass_kernel_spmd(
        nc, in_maps, core_ids=list(range(N_CORES)), trace=want_trace
    )


def _make_in_maps(x, shared, T):
    in_maps = []
    for c in range(N_CORES):
        xs = x[c * B_LOC : (c + 1) * B_LOC, :T, :]  # [B_LOC, T, F]
        xT = np.ascontiguousarray(xs.transpose(2, 1, 0)).astype(np.float16)
        m = {"xT": xT}
        m.update(shared)
        in_maps.append(m)
    return in_maps


def kernel(x, W_i, b_i, W_h, b_h, W_lin, b_lin):
    x = np.asarray(x, np.float32)
    T = x.shape[1]
    key = ("nc", T)
    if key not in _CACHE:
        _CACHE[key] = _build(T)
    nc = _CACHE[key]
    shared = _prep_shared(
        np.asarray(W_i, np.float32), np.asarray(b_i, np.float32),
        np.asarray(W_h, np.float32), np.asarray(b_h, np.float32),
        np.asarray(W_lin, np.float32), np.asarray(b_lin, np.float32),
    )
    in_maps = _make_in_maps(x, shared, T)
    res = _run(nc, in_maps)
    out = np.empty((x.shape[0], N_OUTPUT), np.float32)
    for c in range(N_CORES):
        out[c * B_LOC : (c + 1) * B_LOC] = res.results[c]["y"]
    return out
